# revision 1
# baseline (speedup 1.0000x reference)
"""Equivariant layer block kernel for Trainium2 (8 NeuronCores).

Math: X has shape (A=512, B=512, C=1024) with axes (a, b, c); output
Y (C, B) over (c, d).  The 10 partition terms collapse to:

  Y[c,d] = w2*P_b[d,c] + w3*P_a[d,c] + w4*T[d,c]          (matrix terms)
         + w0*S_ab[c] + w1*D[c]                            (col terms)
         + w7*Q_a[d] + w8*Q_b[d] + w9*QT[d]                (row terms)
         + w5*s + w6*sD                                    (scalar terms)

  P_b[a,c] = sum_b X[a,b,c]      P_a[b,c] = sum_a X[a,b,c]
  T[a,c]   = X[a,a,c]            S_ab[c]  = sum_ab X[a,b,c]
  D[c]     = sum_a T[a,c]        Q_a[a]   = sum_bc X;  Q_b[b] = sum_ac X
  QT[a]    = sum_c T[a,c]        s = sum X;  sD = sum_ac T

Sharding: c (dim 2, 1024) split across 8 cores -> 128 c's per core.
Everything is core-local except the row/scalar terms (pool over c),
which go through a tiny AllReduce.

The per-core shard is passed relaid as x2[a, c, b] (host transpose,
part of sharding prep) so that:
  - P_b = reduce over b is a contiguous innermost DVE reduce;
  - P_a = ones.T @ x2-tile contracts a on partitions with a single-dim
    N=512 moving operand, and each PSUM row [1, 512] is directly a row
    of P_a^T[c, b] -- the exact final layout, no transpose needed.
Matmuls run as float32r (same bits; 1 cycle/col vs 4 for fp32).
The diagonal blocks X[k*128:+128, k*128:+128, :] are passed as xd and
T is DMA-gathered from them on-device (512B contiguous runs).
Big loads alternate between the SP and ACT HWDGE rings to overlap
per-transfer overheads on one ring.
"""

import sys

sys.path.insert(0, "/opt/trn_rl_repo")

import numpy as np

import concourse.bass as bass
import concourse.bacc as bacc
import concourse.tile as tile
from concourse import mybir
from concourse.bass_utils import run_bass_kernel_spmd

F32 = mybir.dt.float32
F32R = mybir.dt.float32r

A = 512  # axis a (input dim 0)
B = 512  # axis b (input dim 1)
C = 1024  # axis c (input dim 2, sharded)
CS = C // 8  # per-core c shard = 128
NAC = 4  # a chunks of 128
NCB = 16  # c blocks per core
CSUB = CS // NCB  # c's per block = 8

_CACHE = {}


def _build() -> bass.Bass:
    nc = bacc.Bacc("TRN2", num_devices=8)
    x2 = nc.dram_tensor("x2", [A, CS, B], F32R, kind="ExternalInput")
    xd = nc.dram_tensor("xd", [NAC, 128, 128, CS], F32, kind="ExternalInput")
    w = nc.dram_tensor("w", [1, 16], F32, kind="ExternalInput")
    y = nc.dram_tensor("y", [CS, B], F32, kind="ExternalOutput")
    eye_d = nc.inline_tensor(np.eye(128, dtype=np.float32), "eye_const")
    cc_in = nc.dram_tensor("cc_in", [1, 1032], F32)
    cc_out = nc.dram_tensor("cc_out", [1, 1032], F32, addr_space="Shared")

    with tile.TileContext(nc) as tc:
        with (
            tc.tile_pool(name="persist", bufs=1) as pp,
            tc.tile_pool(name="xp", bufs=8) as xp,
            tc.tile_pool(name="rp", bufs=4) as rp,
        ):
            # ---- constants / weights ----
            ones_col = pp.tile([128, 1], F32)  # ones on 128 partitions
            nc.gpsimd.memset(ones_col[:], 1.0)
            ones_row = pp.tile([1, 512], F32)  # ones on partition 0
            nc.gpsimd.memset(ones_row[:], 1.0)
            eye_sb = pp.tile([128, 128], F32)
            nc.sync.dma_start(eye_sb[:], eye_d[:])
            wrow = pp.tile([1, 16], F32)
            nc.sync.dma_start(wrow[:], w[:])
            w_sb = pp.tile([128, 16], F32)

            # ---- persistent accumulators ----
            paT2 = pp.tile([128, B], F32)  # P_a^T: [c, b]
            pbAcc = pp.tile([128, NAC, 128], F32)  # P_b: [a', (ac, c)], a = ac*128+a'
            tsb = pp.tile([128, NAC, 128], F32)  # T:   [a', (ac, c)]

            # ---- diagonal T from the xd blocks (512B contiguous runs) ----
            xdf = xd[:].rearrange("k a b c -> (k a b) c")
            for ac in range(NAC):
                st = ac * 128 * 128
                dg = xdf[st : st + 127 * 129 + 1 : 129]
                nc.gpsimd.dma_start(tsb[:, ac, :], dg)

            with tc.tile_pool(name="ps0", bufs=1, space="PSUM") as ps0:
                psw = ps0.tile([128, 16], F32)
                nc.tensor.matmul(
                    psw[:], ones_row[0:1, 0:128], wrow[:], start=True, stop=True
                )
                nc.vector.tensor_copy(w_sb[:], psw[:])

            # ---- main streaming loop over c-blocks ----
            xv = x2[:].rearrange(
                "(ac p) (cb cs) b -> cb ac p cs b", ac=NAC, cs=CSUB
            )
            dma_engines = [nc.sync, nc.scalar]
            with tc.tile_pool(name="psa", bufs=1, space="PSUM") as psa:
                for cb in range(NCB):
                    xts = []
                    for ac in range(NAC):
                        xt = xp.tile([128, CSUB, B], F32R, tag="xt")
                        dma_engines[(cb * NAC + ac) % 2].dma_start(xt[:], xv[cb, ac])
                        xts.append(xt)
                    pgs = [
                        psa.tile([1, B], F32, tag=f"pg{ci}", name=f"pg{ci}_{cb}")
                        for ci in range(CSUB)
                    ]
                    # P_a: ones.T @ X contracts a; PSUM row = P_a^T[c, :]
                    for ac in range(NAC):
                        for ci in range(CSUB):
                            nc.tensor.matmul(
                                pgs[ci][:],
                                ones_col[:].bitcast(F32R),
                                xts[ac][:, ci, :],
                                start=(ac == 0),
                                stop=(ac == NAC - 1),
                                skip_group_check=True,
                            )
                    # evacuate rows c = cb*8+ci of P_a^T.  Compute engines
                    # only address quadrant-aligned partition bases, so go
                    # PSUM -> partition-0 scratch (ACT), then SWDGE DMA
                    # scatters to the target row.
                    for ci in range(CSUB):
                        g = cb * CSUB + ci
                        sc = rp.tile([1, B], F32, tag="evsc", name=f"sc{cb}_{ci}")
                        nc.scalar.copy(sc[:], pgs[ci][:])
                        nc.gpsimd.dma_start(paT2[g : g + 1, :], sc[:])
                    # P_b: contiguous innermost reduce over b, keeps (cs)
                    for ac in range(NAC):
                        nc.vector.reduce_sum(
                            pbAcc[:, ac, cb * CSUB : (cb + 1) * CSUB],
                            xts[ac][:].bitcast(F32),
                            axis=mybir.AxisListType.X,
                        )

            with tc.tile_pool(name="pst", bufs=1, space="PSUM") as pst:
                # ---- row-term partials (feed the AllReduce) ----
                qa = pp.tile([128, NAC], F32)
                qt = pp.tile([128, NAC], F32)
                for ac in range(NAC):
                    nc.vector.reduce_sum(
                        qa[:, ac : ac + 1], pbAcc[:, ac, :], axis=mybir.AxisListType.X
                    )
                    nc.vector.reduce_sum(
                        qt[:, ac : ac + 1], tsb[:, ac, :], axis=mybir.AxisListType.X
                    )
                rq = pp.tile([128, 4], F32)
                rtmp = pp.tile([128, 4], F32)
                nc.vector.tensor_scalar_mul(rq[:], qa[:], w_sb[:, 7:8])
                nc.vector.tensor_scalar_mul(rtmp[:], qt[:], w_sb[:, 9:10])
                nc.vector.tensor_add(rq[:], rq[:], rtmp[:])
                # [128, 4] -> [4, 128] so d = col*128 + part flattens row-major
                psT = pst.tile([4, 128], F32)
                nc.tensor.matmul(psT[:], rq[:], eye_sb[:], is_transpose=True)
                rqT = pp.tile([4, 128], F32)
                nc.vector.tensor_copy(rqT[:], psT[:])
                # Q_b[b] = sum_c P_a^T[c, b]: one partition-reduce matmul
                psQb = pst.tile([1, B], F32)
                nc.tensor.matmul(
                    psQb[:], ones_col[:], paT2[:], start=True, stop=True
                )
                pay_sb = pp.tile([1, 520], F32)
                nc.vector.tensor_scalar_mul(
                    pay_sb[0:1, 0:512], psQb[:], w_sb[0:1, 8:9]
                )

                # ---- col terms S_ab, D and scalar partials ----
                psS = pst.tile([1, 128], F32)
                psD = pst.tile([1, 128], F32)
                for ac in range(NAC):
                    nc.tensor.matmul(
                        psS[:],
                        ones_col[:],
                        pbAcc[:, ac, :],
                        start=(ac == 0),
                        stop=(ac == NAC - 1),
                    )
                    nc.tensor.matmul(
                        psD[:],
                        ones_col[:],
                        tsb[:, ac, :],
                        start=(ac == 0),
                        stop=(ac == NAC - 1),
                    )
                sS = pp.tile([1, 128], F32)
                sD = pp.tile([1, 128], F32)
                nc.vector.tensor_copy(sS[:], psS[:])
                nc.vector.tensor_copy(sD[:], psD[:])
                colrow = pp.tile([1, 128], F32)
                ctmp = pp.tile([1, 128], F32)
                nc.vector.tensor_scalar_mul(colrow[:], sS[:], w_sb[0:1, 0:1])
                nc.vector.tensor_scalar_mul(ctmp[:], sD[:], w_sb[0:1, 1:2])
                nc.vector.tensor_add(colrow[:], colrow[:], ctmp[:])
                red2 = pp.tile([1, 2], F32)
                nc.vector.reduce_sum(red2[0:1, 0:1], sS[:], axis=mybir.AxisListType.X)
                nc.vector.reduce_sum(red2[0:1, 1:2], sD[:], axis=mybir.AxisListType.X)
                nc.vector.memset(pay_sb[0:1, 512:520], 0.0)
                tmp2 = pp.tile([1, 2], F32)
                nc.vector.tensor_scalar_mul(
                    tmp2[0:1, 0:1], red2[0:1, 0:1], w_sb[0:1, 5:6]
                )
                nc.vector.tensor_scalar_mul(
                    tmp2[0:1, 1:2], red2[0:1, 1:2], w_sb[0:1, 6:7]
                )
                nc.vector.tensor_add(
                    pay_sb[0:1, 512:513], tmp2[0:1, 0:1], tmp2[0:1, 1:2]
                )

                # ---- AllReduce payload: w7*Qa+w9*QT | w8*Qb | scalar|pad ----
                nc.gpsimd.dma_start(
                    cc_in[0:1, 0:512].rearrange("r (p f) -> (r p) f", p=4),
                    rqT[:],
                )
                nc.sync.dma_start(cc_in[0:1, 512:1032], pay_sb[:])
                nc.gpsimd.collective_compute(
                    "AllReduce",
                    mybir.AluOpType.add,
                    replica_groups=[list(range(8))],
                    ins=[cc_in[:]],
                    outs=[cc_out[:]],
                )
                rg = pp.tile([1, 1032], F32)
                nc.sync.dma_start(rg[:], cc_out[:])
                row2 = pp.tile([1, 512], F32)
                nc.vector.tensor_add(row2[:], rg[0:1, 0:512], rg[0:1, 512:1024])
                nc.vector.tensor_scalar_add(row2[:], row2[:], rg[0:1, 1024:1025])

                # ---- assemble Y ----
                # PE transpose is a pure permutation move; transpose unscaled
                # into fresh PSUM tiles, weighted-combine on ACT/DVE.  P_a^T
                # is already in final layout.
                ysb = pp.tile([128, 512], F32)
                for ac in range(NAC):
                    psB = pst.tile([128, 128], F32, tag="psB", name=f"psB{ac}")
                    psT2 = pst.tile([128, 128], F32, tag="psT2", name=f"psT2{ac}")
                    nc.tensor.matmul(
                        psB[:], pbAcc[:, ac, :], eye_sb[:], is_transpose=True,
                        start=True, stop=True,
                    )
                    nc.tensor.matmul(
                        psT2[:], tsb[:, ac, :], eye_sb[:], is_transpose=True,
                        start=True, stop=True,
                    )
                    q = ysb[:, ac * 128 : (ac + 1) * 128]
                    tq1 = rp.tile([128, 128], F32, tag="tq1", name=f"tq1_{ac}")
                    tq2 = rp.tile([128, 128], F32, tag="tq2", name=f"tq2_{ac}")
                    tq3 = rp.tile([128, 128], F32, tag="tq3", name=f"tq3_{ac}")
                    nc.scalar.mul(tq1[:], psB[:], w_sb[:, 2:3])
                    nc.vector.tensor_scalar_mul(
                        tq2[:], paT2[:, ac * 128 : (ac + 1) * 128], w_sb[:, 3:4]
                    )
                    nc.scalar.mul(tq3[:], psT2[:], w_sb[:, 4:5])
                    nc.vector.tensor_add(q, tq1[:], tq2[:])
                    nc.vector.tensor_add(q, q, tq3[:])
                # colvec: [1,128] -> [128,1] via 1x1-permutation transpose
                psCV = pst.tile([128, 1], F32)
                nc.tensor.matmul(
                    psCV[:], colrow[:], ones_col[0:1, 0:1], is_transpose=True,
                    start=True, stop=True,
                )
                colv = pp.tile([128, 1], F32)
                nc.vector.tensor_copy(colv[:], psCV[:])
                nc.scalar.add(ysb[:], ysb[:], colv[:, 0:1])
                # row terms + global scalar: broadcast row2 over partitions
                psRow = pst.tile([128, 512], F32)
                nc.tensor.matmul(
                    psRow[:], ones_row[0:1, 0:128], row2[:], start=True, stop=True
                )
                nc.vector.tensor_add(ysb[:], ysb[:], psRow[:])
                nc.sync.dma_start(y[:], ysb[:])
    nc.compile()
    return nc


def _get_nc() -> bass.Bass:
    if "nc" not in _CACHE:
        _CACHE["nc"] = _build()
    return _CACHE["nc"]


def _run(X: np.ndarray, w: np.ndarray, **kwargs):
    nc = _get_nc()
    wpad = np.zeros((1, 16), dtype=np.float32)
    wpad[0, :10] = np.asarray(w, dtype=np.float32).reshape(-1)
    X = np.asarray(X, dtype=np.float32)
    XT = np.ascontiguousarray(X.transpose(0, 2, 1))  # (a, c, b)
    xd_full = np.stack(
        [X[k * 128 : (k + 1) * 128, k * 128 : (k + 1) * 128, :] for k in range(NAC)]
    )
    in_maps = []
    for k in range(8):
        x2 = np.ascontiguousarray(XT[:, k * CS : (k + 1) * CS, :])
        xdk = np.ascontiguousarray(xd_full[:, :, :, k * CS : (k + 1) * CS])
        in_maps.append({"x2": x2, "xd": xdk, "w": wpad})
    res = run_bass_kernel_spmd(nc, in_maps, core_ids=list(range(8)), **kwargs)
    Y = np.concatenate([r["y"] for r in res.results], axis=0)
    return Y, res


def kernel(X: np.ndarray, weights: np.ndarray) -> np.ndarray:
    X = np.asarray(X, dtype=np.float32)
    Y, _ = _run(X, weights)
    return Y



# revision 2
# speedup vs baseline: 1.0424x; 1.0424x over previous
"""Equivariant layer block kernel for Trainium2 (8 NeuronCores), v2.

Math: X has shape (A=512, B=512, C=1024) with axes (a, b, c); output
Y (C, B) over (c, d).  The 10 partition terms collapse to:

  Y[c,d] = w2*P_b[d,c] + w3*P_a[d,c] + w4*T[d,c]          (matrix terms)
         + w0*S_ab[c] + w1*D[c]                            (col terms)
         + w7*Q_a[d] + w8*Q_b[d] + w9*QT[d]                (row terms)
         + w5*s + w6*sD                                    (scalar terms)

  P_b[a,c] = sum_b X[a,b,c]      P_a[b,c] = sum_a X[a,b,c]
  T[a,c]   = X[a,a,c]            S_ab[c]  = sum_ab X[a,b,c]
  D[c]     = sum_a T[a,c]        Q_a[a]   = sum_bc X;  Q_b[b] = sum_ac X
  QT[a]    = sum_c T[a,c]        s = sum X;  sD = sum_ac T

Sharding: c (dim 2, 1024) split across 8 cores -> 128 c's per core.
Everything is core-local except the row/scalar terms (pool over c),
which go through a tiny AllReduce.

v2 strategy (vs the f32 baseline): X is cast to fp16 on the host,
halving HBM traffic (67 MB/core, ~187us DMA floor at 358 GB/s).  The
f32 baseline hid a 273us DVE reduce_sum (1x mode) under 375us of DMA;
that no longer fits, so the reductions are restructured:

  - Layout x2[a, b, cs] (cs innermost).  DMA tiles [128a, 64b, 128cs]
    are 16KB-contiguous per partition.
  - P_a (contract a, on partitions): ones-at-column-m matmuls, one per
    b-quad (N=512 = 4b x 128cs), all accumulating into a single
    resident PSUM bank pa_ps[128 quads, 512]; one DVE evacuation.
  - P_b (contract b, free axis): split across engines.  For PE_SLABS
    b-slabs the PE does identity-matmul PSUM accumulation over b
    (N=128/mm); the remaining slabs use a DVE pairwise-halving tree
    (fp16 tensor_tensor runs 2x mode vs 1x for reduce_sum).
  - Tail: w2*P_b + w4*T fused on DVE, PE-transposed into a PSUM tile
    that also accumulates the broadcast row/col terms; final
    Y = (paT * w3) + psum via one scalar_tensor_tensor.
"""

import sys

sys.path.insert(0, "/opt/trn_rl_repo")

import numpy as np

import concourse.bass as bass
import concourse.bacc as bacc
import concourse.tile as tile
from concourse import mybir
from concourse.bass_utils import run_bass_kernel_spmd

F32 = mybir.dt.float32
F32R = mybir.dt.float32r
F16 = mybir.dt.float16

A = 512  # axis a (input dim 0)
B = 512  # axis b (input dim 1)
C = 1024  # axis c (input dim 2, sharded)
CS = C // 8  # per-core c shard = 128
NAC = 4  # a chunks of 128
NBS = 8  # b slabs
BS = B // NBS  # b's per slab = 64
NQ = BS // 4  # b-quads per slab = 16
PE_SLABS = 2  # slabs whose P_b runs on the PE; the rest go to the DVE tree

_CACHE = {}


def _build() -> bass.Bass:
    nc = bacc.Bacc("TRN2", num_devices=8)
    x2 = nc.dram_tensor("x2", [A, B, CS], F16, kind="ExternalInput")
    td = nc.dram_tensor("td", [128, NAC, CS], F32, kind="ExternalInput")
    w = nc.dram_tensor("w", [1, 16], F32, kind="ExternalInput")
    y = nc.dram_tensor("y", [CS, B], F32, kind="ExternalOutput")
    eye_d = nc.inline_tensor(np.eye(128, dtype=np.float32), "eye_const")
    eyeh_d = nc.inline_tensor(np.eye(128, dtype=np.float16), "eyeh_const")
    # onesat[:, m*32+j] = (j == m): slice [:, m*32:(m+1)*32] is the
    # [128, 32] matrix with an all-ones column at position m.
    onesat_np = np.broadcast_to(np.eye(32, dtype=np.float16), (128, 32, 32))
    onesat_d = nc.inline_tensor(
        np.ascontiguousarray(onesat_np).reshape(128, 32 * 32), "onesat_const"
    )
    cc_in = nc.dram_tensor("cc_in", [1, 1032], F32)
    cc_out = nc.dram_tensor("cc_out", [1, 1032], F32, addr_space="Shared")

    with tile.TileContext(nc) as tc:
        with (
            tc.tile_pool(name="persist", bufs=1) as pp,
            tc.tile_pool(name="xp", bufs=4) as xp,
            tc.tile_pool(name="tp", bufs=2) as tp,
            tc.tile_pool(name="rp", bufs=2) as rp,
        ):
            # ---- constants / weights ----
            ones_col = pp.tile([128, 1], F32)
            nc.gpsimd.memset(ones_col[:], 1.0)
            ones_row = pp.tile([1, 512], F32)
            nc.gpsimd.memset(ones_row[:], 1.0)
            eye_sb = pp.tile([128, 128], F32)
            nc.sync.dma_start(eye_sb[:], eye_d[:])
            eyeh_sb = pp.tile([128, 128], F16)
            nc.sync.dma_start(eyeh_sb[:], eyeh_d[:])
            onesat_sb = pp.tile([128, 32 * 32], F16)
            nc.sync.dma_start(onesat_sb[:], onesat_d[:])
            wrow = pp.tile([1, 16], F32)
            nc.sync.dma_start(wrow[:], w[:])
            w_sb = pp.tile([128, 16], F32)

            # ---- persistent accumulators ----
            paT = pp.tile([128, B], F32)  # P_a^T: [cs, b]
            pa_sb = pp.tile([128, B], F32)  # P_a:   [quad, (b%4, cs)]
            pbAcc = pp.tile([128, NAC, CS], F32)  # P_b: [a', (ac, cs)]
            tsb = pp.tile([128, NAC, CS], F32)  # T:   [a', (ac, cs)]
            nc.sync.dma_start(tsb[:], td[:])

            with tc.tile_pool(name="ps0", bufs=1, space="PSUM") as ps0:
                psw = ps0.tile([128, 16], F32)
                nc.tensor.matmul(
                    psw[:], ones_row[0:1, 0:128], wrow[:], start=True, stop=True
                )
                nc.vector.tensor_copy(w_sb[:], psw[:])

            # ---- main streaming loop over (b-slab, a-chunk) tiles ----
            xv = x2[:].rearrange(
                "(ac p) (bs bsub) cs -> bs ac p bsub cs", p=128, bsub=BS
            )
            dma_engines = [nc.sync, nc.scalar]
            dve_first = {}  # ac -> first DVE slab (writes pbAcc directly)
            with tc.tile_pool(name="psa", bufs=1, space="PSUM") as psa:
                pa_ps = psa.tile([128, 512], F32)
                pb_ps = [
                    psa.tile([128, CS], F32, tag=f"pbps{ac}", name=f"pbps{ac}")
                    for ac in range(NAC)
                ]
                for bs in range(NBS):
                    for ac in range(NAC):
                        xt = xp.tile([128, BS, CS], F16, tag="xt")
                        dma_engines[(bs * NAC + ac) % 2].dma_start(
                            xt[:], xv[bs, ac]
                        )
                        # P_a: one ones-at-column matmul per b-quad; row
                        # m = global quad index within its 32-row group.
                        for qq in range(NQ):
                            q = bs * NQ + qq
                            g, m = divmod(q, 32)
                            nc.tensor.matmul(
                                pa_ps[32 * g : 32 * g + 32, :],
                                onesat_sb[:, 32 * m : 32 * m + 32],
                                xt[:, 4 * qq : 4 * qq + 4, :],
                                start=(ac == 0 and qq == 0 and bs == 2 * g),
                                stop=(ac == NAC - 1 and qq == NQ - 1
                                      and bs == 2 * g + 1),
                                skip_group_check=True,
                                tile_position=(0, 32 * g),
                            )
                        if bs < PE_SLABS:
                            # P_b on PE: identity matmuls accumulate over b
                            for j in range(BS):
                                nc.tensor.matmul(
                                    pb_ps[ac][:],
                                    eyeh_sb[:],
                                    xt[:, j, :],
                                    start=(bs == 0 and j == 0),
                                    stop=(bs == PE_SLABS - 1 and j == BS - 1),
                                    skip_group_check=True,
                                )
                        else:
                            # P_b on DVE: pairwise-halving tree over b
                            cur, width = xt, BS
                            while width > 2:
                                h = width // 2
                                nxt = tp.tile(
                                    [128, h, CS], F16, tag=f"tr{h}",
                                    name=f"tr{h}_{bs}_{ac}",
                                )
                                nc.vector.tensor_add(
                                    nxt[:], cur[:, 0:h, :], cur[:, h:width, :]
                                )
                                cur, width = nxt, h
                            dst = pbAcc[:, ac, :]
                            if ac not in dve_first:
                                dve_first[ac] = bs
                                nc.vector.tensor_add(
                                    dst, cur[:, 0, :], cur[:, 1, :]
                                )
                            else:
                                fin = tp.tile(
                                    [128, CS], F16, tag="fin",
                                    name=f"fin_{bs}_{ac}",
                                )
                                nc.vector.tensor_add(
                                    fin[:], cur[:, 0, :], cur[:, 1, :]
                                )
                                nc.vector.tensor_add(dst, dst, fin[:])

                # ---- evacuate stream PSUM state ----
                nc.vector.tensor_copy(pa_sb[:], pa_ps[:])
                for ac in range(NAC):
                    nc.vector.tensor_add(
                        pbAcc[:, ac, :], pbAcc[:, ac, :], pb_ps[ac][:]
                    )

            with tc.tile_pool(name="pst", bufs=1, space="PSUM") as pst:
                # ---- row-term partials (feed the AllReduce) ----
                qa = pp.tile([128, NAC], F32)
                qt = pp.tile([128, NAC], F32)
                for ac in range(NAC):
                    nc.vector.reduce_sum(
                        qa[:, ac : ac + 1], pbAcc[:, ac, :],
                        axis=mybir.AxisListType.X,
                    )
                    nc.vector.reduce_sum(
                        qt[:, ac : ac + 1], tsb[:, ac, :],
                        axis=mybir.AxisListType.X,
                    )
                rq = pp.tile([128, 4], F32)
                rtmp = pp.tile([128, 4], F32)
                nc.vector.tensor_scalar_mul(rq[:], qa[:], w_sb[:, 7:8])
                nc.vector.tensor_scalar_mul(rtmp[:], qt[:], w_sb[:, 9:10])
                nc.vector.tensor_add(rq[:], rq[:], rtmp[:])
                # [128, 4] -> [4, 128] so a = col*128 + part flattens row-major
                psT = pst.tile([4, 128], F32)
                nc.tensor.matmul(psT[:], rq[:], eye_sb[:], is_transpose=True)
                rqT = pp.tile([4, 128], F32)
                nc.vector.tensor_copy(rqT[:], psT[:])

                # ---- P_a transposes: pa_sb[quad, (j, cs)] -> paT[cs, b] ----
                pav = pa_sb[:].rearrange("p (j cs) -> p j cs", j=4)
                for j in range(4):
                    pstj = pst.tile(
                        [128, 128], F32, tag="pstj", name=f"pstj{j}"
                    )
                    nc.tensor.matmul(
                        pstj[:], pav[:, j, :], eye_sb[:], is_transpose=True,
                        start=True, stop=True,
                    )
                    # b = 4*quad + j -> strided columns of paT
                    nc.vector.tensor_copy(
                        paT[:].rearrange("p (q j) -> p q j", j=4)[:, :, j],
                        pstj[:],
                    )

                # Q_b[b] = sum_cs P_a^T[cs, b]: one partition-reduce matmul
                psQb = pst.tile([1, B], F32)
                nc.tensor.matmul(
                    psQb[:], ones_col[:], paT[:], start=True, stop=True,
                )
                pay_sb = pp.tile([1, 520], F32)
                nc.vector.tensor_scalar_mul(
                    pay_sb[0:1, 0:512], psQb[:], w_sb[0:1, 8:9]
                )

                # ---- col terms S_ab, D and scalar partials ----
                sS = pp.tile([128, 1], F32)
                nc.vector.reduce_sum(sS[:], paT[:], axis=mybir.AxisListType.X)
                psD = pst.tile([1, 128], F32)
                for ac in range(NAC):
                    nc.tensor.matmul(
                        psD[:],
                        ones_col[:],
                        tsb[:, ac, :],
                        start=(ac == 0),
                        stop=(ac == NAC - 1),
                    )
                sD = pp.tile([1, 128], F32)
                nc.vector.tensor_copy(sD[:], psD[:])
                # colrow[0, cs] = w0*S + w1*D  (row layout, becomes a
                # broadcast-add over free via ones_row matmul later)
                sSrowp = pst.tile([1, 128], F32)
                nc.tensor.matmul(
                    sSrowp[:], sS[:], eye_sb[:], is_transpose=True,
                    start=True, stop=True,
                )
                sSrow = pp.tile([1, 128], F32)
                nc.vector.tensor_copy(sSrow[:], sSrowp[:])
                colrow = pp.tile([1, 128], F32)
                ctmp = pp.tile([1, 128], F32)
                nc.vector.tensor_scalar_mul(colrow[:], sSrow[:], w_sb[0:1, 0:1])
                nc.vector.tensor_scalar_mul(ctmp[:], sD[:], w_sb[0:1, 1:2])
                nc.vector.tensor_add(colrow[:], colrow[:], ctmp[:])
                red2 = pp.tile([1, 2], F32)
                nc.vector.reduce_sum(
                    red2[0:1, 0:1], sSrow[:], axis=mybir.AxisListType.X
                )
                nc.vector.reduce_sum(
                    red2[0:1, 1:2], sD[:], axis=mybir.AxisListType.X
                )
                nc.vector.memset(pay_sb[0:1, 512:520], 0.0)
                tmp2 = pp.tile([1, 2], F32)
                nc.vector.tensor_scalar_mul(
                    tmp2[0:1, 0:1], red2[0:1, 0:1], w_sb[0:1, 5:6]
                )
                nc.vector.tensor_scalar_mul(
                    tmp2[0:1, 1:2], red2[0:1, 1:2], w_sb[0:1, 6:7]
                )
                nc.vector.tensor_add(
                    pay_sb[0:1, 512:513], tmp2[0:1, 0:1], tmp2[0:1, 1:2]
                )

                # ---- AllReduce payload: w7*Qa+w9*QT | w8*Qb | scalar|pad ----
                nc.gpsimd.dma_start(
                    cc_in[0:1, 0:512].rearrange("r (p f) -> (r p) f", p=4),
                    rqT[:],
                )
                nc.sync.dma_start(cc_in[0:1, 512:1032], pay_sb[:])
                nc.gpsimd.collective_compute(
                    "AllReduce",
                    mybir.AluOpType.add,
                    replica_groups=[list(range(8))],
                    ins=[cc_in[:]],
                    outs=[cc_out[:]],
                )
                rg = pp.tile([1, 1032], F32)
                nc.sync.dma_start(rg[:], cc_out[:])
                row2 = pp.tile([1, 512], F32)
                nc.vector.tensor_add(row2[:], rg[0:1, 0:512], rg[0:1, 512:1024])
                nc.vector.tensor_scalar_add(row2[:], row2[:], rg[0:1, 1024:1025])

                # ---- assemble Y ----
                # tmp = w2*P_b + w4*T in [a', (ac, cs)] layout, then 4 PE
                # transposes accumulate into psum_final on top of the
                # broadcast row/col terms.
                tsw = pp.tile([128, NAC, CS], F32)
                nc.scalar.mul(tsw[:], tsb[:], w_sb[:, 4:5])
                tmp = pp.tile([128, NAC, CS], F32)
                nc.vector.scalar_tensor_tensor(
                    tmp[:], pbAcc[:], w_sb[:, 2:3], tsw[:],
                    op0=mybir.AluOpType.mult, op1=mybir.AluOpType.add,
                )
                psF = pst.tile([128, 512], F32)
                # row terms (broadcast row2 over partitions)
                nc.tensor.matmul(
                    psF[:], ones_row[0:1, 0:128], row2[:],
                    start=True, stop=True, skip_group_check=True,
                )
                # col terms (broadcast colrow over free)
                nc.tensor.matmul(
                    psF[:], colrow[:], ones_row[:],
                    start=False, stop=False, skip_group_check=True,
                )
                for ac in range(NAC):
                    nc.tensor.matmul(
                        psF[:, 128 * ac : 128 * ac + 128],
                        tmp[:, ac, :],
                        eye_sb[:],
                        is_transpose=True,
                        start=False, stop=(ac == NAC - 1),
                        skip_group_check=True,
                    )
                ysb = pp.tile([128, 512], F32)
                nc.vector.scalar_tensor_tensor(
                    ysb[:], paT[:], w_sb[:, 3:4], psF[:],
                    op0=mybir.AluOpType.mult, op1=mybir.AluOpType.add,
                )
                nc.sync.dma_start(y[:], ysb[:])
    nc.compile()
    return nc


def _get_nc() -> bass.Bass:
    if "nc" not in _CACHE:
        _CACHE["nc"] = _build()
    return _CACHE["nc"]


def _run(X: np.ndarray, w: np.ndarray, **kwargs):
    nc = _get_nc()
    wpad = np.zeros((1, 16), dtype=np.float32)
    wpad[0, :10] = np.asarray(w, dtype=np.float32).reshape(-1)
    X = np.asarray(X, dtype=np.float32)
    Xh = X.astype(np.float16)  # (a, b, c); cs innermost stays contiguous
    idx = np.arange(A)
    diag = X[idx, idx, :]  # (512, 1024) f32
    # td[a', ac, cs] with a = ac*128 + a'
    td_full = np.ascontiguousarray(diag.reshape(NAC, 128, C).transpose(1, 0, 2))
    in_maps = []
    for k in range(8):
        sl = slice(k * CS, (k + 1) * CS)
        in_maps.append({
            "x2": np.ascontiguousarray(Xh[:, :, sl]),
            "td": np.ascontiguousarray(td_full[:, :, sl]),
            "w": wpad,
        })
    res = run_bass_kernel_spmd(nc, in_maps, core_ids=list(range(8)), **kwargs)
    Y = np.concatenate([r["y"] for r in res.results], axis=0)
    return Y, res


def kernel(X: np.ndarray, weights: np.ndarray) -> np.ndarray:
    X = np.asarray(X, dtype=np.float32)
    Y, _ = _run(X, weights)
    return Y


# revision 3
# speedup vs baseline: 1.0768x; 1.0331x over previous
"""Equivariant layer block kernel for Trainium2 (8 NeuronCores), v2.

Math: X has shape (A=512, B=512, C=1024) with axes (a, b, c); output
Y (C, B) over (c, d).  The 10 partition terms collapse to:

  Y[c,d] = w2*P_b[d,c] + w3*P_a[d,c] + w4*T[d,c]          (matrix terms)
         + w0*S_ab[c] + w1*D[c]                            (col terms)
         + w7*Q_a[d] + w8*Q_b[d] + w9*QT[d]                (row terms)
         + w5*s + w6*sD                                    (scalar terms)

  P_b[a,c] = sum_b X[a,b,c]      P_a[b,c] = sum_a X[a,b,c]
  T[a,c]   = X[a,a,c]            S_ab[c]  = sum_ab X[a,b,c]
  D[c]     = sum_a T[a,c]        Q_a[a]   = sum_bc X;  Q_b[b] = sum_ac X
  QT[a]    = sum_c T[a,c]        s = sum X;  sD = sum_ac T

Sharding: c (dim 2, 1024) split across 8 cores -> 128 c's per core.
Everything is core-local except the row/scalar terms (pool over c),
which go through a tiny AllReduce.

v2 strategy (vs the f32 baseline): X is cast to fp16 on the host,
halving HBM traffic (67 MB/core, ~187us DMA floor at 358 GB/s).  The
f32 baseline hid a 273us DVE reduce_sum (1x mode) under 375us of DMA;
that no longer fits, so the reductions are restructured:

  - Layout x2[a, b, cs] (cs innermost).  DMA tiles [128a, 64b, 128cs]
    are 16KB-contiguous per partition.
  - P_a (contract a, on partitions): ones-at-column-m matmuls, one per
    b-quad (N=512 = 4b x 128cs), all accumulating into a single
    resident PSUM bank pa_ps[128 quads, 512]; one DVE evacuation.
  - P_b (contract b, free axis): split across engines.  For PE_SLABS
    b-slabs the PE does identity-matmul PSUM accumulation over b
    (N=128/mm); the remaining slabs use a DVE pairwise-halving tree
    (fp16 tensor_tensor runs 2x mode vs 1x for reduce_sum).
  - Tail: w2*P_b + w4*T fused on DVE, PE-transposed into a PSUM tile
    that also accumulates the broadcast row/col terms; final
    Y = (paT * w3) + psum via one scalar_tensor_tensor.
"""

import sys

sys.path.insert(0, "/opt/trn_rl_repo")

import numpy as np

import concourse.bass as bass
import concourse.bacc as bacc
import concourse.tile as tile
from concourse import mybir
from concourse.bass_utils import run_bass_kernel_spmd

F32 = mybir.dt.float32
F32R = mybir.dt.float32r
F16 = mybir.dt.float16

A = 512  # axis a (input dim 0)
B = 512  # axis b (input dim 1)
C = 1024  # axis c (input dim 2, sharded)
CS = C // 8  # per-core c shard = 128
NAC = 4  # a chunks of 128
NBS = 8  # b slabs
BS = B // NBS  # b's per slab = 64
NQ = BS // 4  # b-quads per slab = 16
PE_SLABS = 2  # slabs whose P_b runs on the PE; the rest go to the DVE tree

_CACHE = {}


def _build() -> bass.Bass:
    nc = bacc.Bacc("TRN2", num_devices=8)
    x2 = nc.dram_tensor("x2", [A, B, CS], F16, kind="ExternalInput")
    td = nc.dram_tensor("td", [128, NAC, CS], F32, kind="ExternalInput")
    w = nc.dram_tensor("w", [1, 16], F32, kind="ExternalInput")
    y = nc.dram_tensor("y", [CS, B], F32, kind="ExternalOutput")
    eye_d = nc.inline_tensor(np.eye(128, dtype=np.float32), "eye_const")
    eyeh_d = nc.inline_tensor(np.eye(128, dtype=np.float16), "eyeh_const")
    # onesat[:, m*32+j] = (j == m): slice [:, m*32:(m+1)*32] is the
    # [128, 32] matrix with an all-ones column at position m.
    onesat_np = np.broadcast_to(np.eye(32, dtype=np.float16), (128, 32, 32))
    onesat_d = nc.inline_tensor(
        np.ascontiguousarray(onesat_np).reshape(128, 32 * 32), "onesat_const"
    )
    cc_in = nc.dram_tensor("cc_in", [1, 1032], F32)
    cc_out = nc.dram_tensor("cc_out", [1, 1032], F32, addr_space="Shared")

    with tile.TileContext(nc) as tc:
        with (
            tc.tile_pool(name="persist", bufs=1) as pp,
            tc.tile_pool(name="xp", bufs=6) as xp,
            tc.tile_pool(name="tp", bufs=2) as tp,
            tc.tile_pool(name="rp", bufs=2) as rp,
        ):
            # ---- constants / weights ----
            ones_col = pp.tile([128, 1], F32)
            nc.gpsimd.memset(ones_col[:], 1.0)
            ones_row = pp.tile([1, 512], F32)
            nc.gpsimd.memset(ones_row[:], 1.0)
            eye_sb = pp.tile([128, 128], F32)
            nc.gpsimd.dma_start(eye_sb[:], eye_d[:])
            eyeh_sb = pp.tile([128, 128], F16)
            nc.gpsimd.dma_start(eyeh_sb[:], eyeh_d[:])
            onesat_sb = pp.tile([128, 32 * 32], F16)
            nc.gpsimd.dma_start(onesat_sb[:], onesat_d[:])
            wrow = pp.tile([1, 16], F32)
            nc.gpsimd.dma_start(wrow[:], w[:])
            w_sb = pp.tile([128, 16], F32)

            # ---- persistent accumulators ----
            paT = pp.tile([128, B], F32)  # P_a^T: [cs, b]
            pa_sb = pp.tile([128, B], F32)  # P_a:   [quad, (b%4, cs)]
            pbAcc = pp.tile([128, NAC, CS], F32)  # P_b: [a', (ac, cs)]
            tsb = pp.tile([128, NAC, CS], F32)  # T:   [a', (ac, cs)]
            nc.gpsimd.dma_start(tsb[:], td[:])

            with tc.tile_pool(name="ps0", bufs=1, space="PSUM") as ps0:
                psw = ps0.tile([128, 16], F32)
                nc.tensor.matmul(
                    psw[:], ones_row[0:1, 0:128], wrow[:], start=True, stop=True
                )
                nc.vector.tensor_copy(w_sb[:], psw[:])

            # ---- main streaming loop over (b-slab, a-chunk) tiles ----
            xv = x2[:].rearrange(
                "(ac p) (bs bsub) cs -> bs ac p bsub cs", p=128, bsub=BS
            )
            dma_engines = [nc.sync, nc.scalar]
            dve_first = {}  # ac -> first DVE slab (writes pbAcc directly)
            with tc.tile_pool(name="psa", bufs=1, space="PSUM") as psa:
                pa_ps = psa.tile([128, 512], F32)
                pb_ps = [
                    psa.tile([128, CS], F32, tag=f"pbps{ac}", name=f"pbps{ac}")
                    for ac in range(NAC)
                ]
                for bs in range(NBS):
                    for ac in range(NAC):
                        xt = xp.tile([128, BS, CS], F16, tag="xt")
                        dma_engines[(bs * NAC + ac) % 2].dma_start(
                            xt[:], xv[bs, ac]
                        )
                        # P_a: one ones-at-column matmul per b-quad; row
                        # m = global quad index within its 32-row group.
                        for qq in range(NQ):
                            q = bs * NQ + qq
                            g, m = divmod(q, 32)
                            nc.tensor.matmul(
                                pa_ps[32 * g : 32 * g + 32, :],
                                onesat_sb[:, 32 * m : 32 * m + 32],
                                xt[:, 4 * qq : 4 * qq + 4, :],
                                start=(ac == 0 and qq == 0 and bs == 2 * g),
                                stop=(ac == NAC - 1 and qq == NQ - 1
                                      and bs == 2 * g + 1),
                                skip_group_check=True,
                                tile_position=(0, 32 * g),
                            )
                        if bs < PE_SLABS:
                            # P_b on PE: identity matmuls accumulate over b
                            for j in range(BS):
                                nc.tensor.matmul(
                                    pb_ps[ac][:],
                                    eyeh_sb[:],
                                    xt[:, j, :],
                                    start=(bs == 0 and j == 0),
                                    stop=(bs == PE_SLABS - 1 and j == BS - 1),
                                    skip_group_check=True,
                                )
                        else:
                            # P_b on DVE: pairwise-halving tree over b
                            cur, width = xt, BS
                            while width > 2:
                                h = width // 2
                                nxt = tp.tile(
                                    [128, h, CS], F16, tag=f"tr{h}",
                                    name=f"tr{h}_{bs}_{ac}",
                                )
                                nc.vector.tensor_add(
                                    nxt[:], cur[:, 0:h, :], cur[:, h:width, :]
                                )
                                cur, width = nxt, h
                            dst = pbAcc[:, ac, :]
                            if ac not in dve_first:
                                dve_first[ac] = bs
                                nc.vector.tensor_add(
                                    dst, cur[:, 0, :], cur[:, 1, :]
                                )
                            else:
                                fin = tp.tile(
                                    [128, CS], F16, tag="fin",
                                    name=f"fin_{bs}_{ac}",
                                )
                                nc.vector.tensor_add(
                                    fin[:], cur[:, 0, :], cur[:, 1, :]
                                )
                                nc.vector.tensor_add(dst, dst, fin[:])

                # ---- evacuate stream PSUM state ----
                nc.vector.tensor_copy(pa_sb[:], pa_ps[:])
                for ac in range(NAC):
                    nc.vector.tensor_add(
                        pbAcc[:, ac, :], pbAcc[:, ac, :], pb_ps[ac][:]
                    )

            with tc.tile_pool(name="pst", bufs=1, space="PSUM") as pst:
                # ---- row-term partials (feed the AllReduce) ----
                qa = pp.tile([128, NAC], F32)
                qt = pp.tile([128, NAC], F32)
                for ac in range(NAC):
                    nc.vector.reduce_sum(
                        qa[:, ac : ac + 1], pbAcc[:, ac, :],
                        axis=mybir.AxisListType.X,
                    )
                    nc.vector.reduce_sum(
                        qt[:, ac : ac + 1], tsb[:, ac, :],
                        axis=mybir.AxisListType.X,
                    )
                rq = pp.tile([128, 4], F32)
                rtmp = pp.tile([128, 4], F32)
                nc.vector.tensor_scalar_mul(rq[:], qa[:], w_sb[:, 7:8])
                nc.vector.tensor_scalar_mul(rtmp[:], qt[:], w_sb[:, 9:10])
                nc.vector.tensor_add(rq[:], rq[:], rtmp[:])
                # [128, 4] -> [4, 128] so a = col*128 + part flattens row-major
                psT = pst.tile([4, 128], F32)
                nc.tensor.matmul(psT[:], rq[:], eye_sb[:], is_transpose=True)
                rqT = pp.tile([4, 128], F32)
                nc.vector.tensor_copy(rqT[:], psT[:])

                # ---- P_a transposes: pa_sb[quad, (j, cs)] -> paT[cs, b] ----
                pav = pa_sb[:].rearrange("p (j cs) -> p j cs", j=4)
                for j in range(4):
                    pstj = pst.tile(
                        [128, 128], F32, tag="pstj", name=f"pstj{j}"
                    )
                    nc.tensor.matmul(
                        pstj[:], pav[:, j, :], eye_sb[:], is_transpose=True,
                        start=True, stop=True,
                    )
                    # b = 4*quad + j -> strided columns of paT
                    nc.vector.tensor_copy(
                        paT[:].rearrange("p (q j) -> p q j", j=4)[:, :, j],
                        pstj[:],
                    )

                # Q_b[b] = sum_cs P_a^T[cs, b]: one partition-reduce matmul
                psQb = pst.tile([1, B], F32)
                nc.tensor.matmul(
                    psQb[:], ones_col[:], paT[:], start=True, stop=True,
                )
                pay_sb = pp.tile([1, 520], F32)
                nc.vector.tensor_scalar_mul(
                    pay_sb[0:1, 0:512], psQb[:], w_sb[0:1, 8:9]
                )

                # ---- col terms S_ab, D and scalar partials ----
                sS = pp.tile([128, 1], F32)
                nc.vector.reduce_sum(sS[:], paT[:], axis=mybir.AxisListType.X)
                psD = pst.tile([1, 128], F32)
                for ac in range(NAC):
                    nc.tensor.matmul(
                        psD[:],
                        ones_col[:],
                        tsb[:, ac, :],
                        start=(ac == 0),
                        stop=(ac == NAC - 1),
                    )
                sD = pp.tile([1, 128], F32)
                nc.vector.tensor_copy(sD[:], psD[:])
                # colrow[0, cs] = w0*S + w1*D  (row layout, becomes a
                # broadcast-add over free via ones_row matmul later)
                sSrowp = pst.tile([1, 128], F32)
                nc.tensor.matmul(
                    sSrowp[:], sS[:], eye_sb[:], is_transpose=True,
                    start=True, stop=True,
                )
                sSrow = pp.tile([1, 128], F32)
                nc.vector.tensor_copy(sSrow[:], sSrowp[:])
                colrow = pp.tile([1, 128], F32)
                ctmp = pp.tile([1, 128], F32)
                nc.vector.tensor_scalar_mul(colrow[:], sSrow[:], w_sb[0:1, 0:1])
                nc.vector.tensor_scalar_mul(ctmp[:], sD[:], w_sb[0:1, 1:2])
                nc.vector.tensor_add(colrow[:], colrow[:], ctmp[:])
                red2 = pp.tile([1, 2], F32)
                nc.vector.reduce_sum(
                    red2[0:1, 0:1], sSrow[:], axis=mybir.AxisListType.X
                )
                nc.vector.reduce_sum(
                    red2[0:1, 1:2], sD[:], axis=mybir.AxisListType.X
                )
                nc.vector.memset(pay_sb[0:1, 512:520], 0.0)
                tmp2 = pp.tile([1, 2], F32)
                nc.vector.tensor_scalar_mul(
                    tmp2[0:1, 0:1], red2[0:1, 0:1], w_sb[0:1, 5:6]
                )
                nc.vector.tensor_scalar_mul(
                    tmp2[0:1, 1:2], red2[0:1, 1:2], w_sb[0:1, 6:7]
                )
                nc.vector.tensor_add(
                    pay_sb[0:1, 512:513], tmp2[0:1, 0:1], tmp2[0:1, 1:2]
                )

                # ---- AllReduce payload: w7*Qa+w9*QT | w8*Qb | scalar|pad ----
                nc.gpsimd.dma_start(
                    cc_in[0:1, 0:512].rearrange("r (p f) -> (r p) f", p=4),
                    rqT[:],
                )
                nc.sync.dma_start(cc_in[0:1, 512:1032], pay_sb[:])
                nc.gpsimd.collective_compute(
                    "AllReduce",
                    mybir.AluOpType.add,
                    replica_groups=[list(range(8))],
                    ins=[cc_in[:]],
                    outs=[cc_out[:]],
                )
                rg = pp.tile([1, 1032], F32)
                nc.sync.dma_start(rg[:], cc_out[:])
                row2 = pp.tile([1, 512], F32)
                nc.vector.tensor_add(row2[:], rg[0:1, 0:512], rg[0:1, 512:1024])
                nc.vector.tensor_scalar_add(row2[:], row2[:], rg[0:1, 1024:1025])

                # ---- assemble Y ----
                # tmp = w2*P_b + w4*T in [a', (ac, cs)] layout, then 4 PE
                # transposes accumulate into psum_final on top of the
                # broadcast row/col terms.
                tsw = pp.tile([128, NAC, CS], F32)
                nc.scalar.mul(tsw[:], tsb[:], w_sb[:, 4:5])
                tmp = pp.tile([128, NAC, CS], F32)
                nc.vector.scalar_tensor_tensor(
                    tmp[:], pbAcc[:], w_sb[:, 2:3], tsw[:],
                    op0=mybir.AluOpType.mult, op1=mybir.AluOpType.add,
                )
                psF = pst.tile([128, 512], F32)
                # row terms (broadcast row2 over partitions)
                nc.tensor.matmul(
                    psF[:], ones_row[0:1, 0:128], row2[:],
                    start=True, stop=True, skip_group_check=True,
                )
                # col terms (broadcast colrow over free)
                nc.tensor.matmul(
                    psF[:], colrow[:], ones_row[:],
                    start=False, stop=False, skip_group_check=True,
                )
                for ac in range(NAC):
                    nc.tensor.matmul(
                        psF[:, 128 * ac : 128 * ac + 128],
                        tmp[:, ac, :],
                        eye_sb[:],
                        is_transpose=True,
                        start=False, stop=(ac == NAC - 1),
                        skip_group_check=True,
                    )
                ysb = pp.tile([128, 512], F32)
                nc.vector.scalar_tensor_tensor(
                    ysb[:], paT[:], w_sb[:, 3:4], psF[:],
                    op0=mybir.AluOpType.mult, op1=mybir.AluOpType.add,
                )
                nc.sync.dma_start(y[:], ysb[:])
    nc.compile()
    return nc


def _get_nc() -> bass.Bass:
    if "nc" not in _CACHE:
        _CACHE["nc"] = _build()
    return _CACHE["nc"]


def _run(X: np.ndarray, w: np.ndarray, **kwargs):
    nc = _get_nc()
    wpad = np.zeros((1, 16), dtype=np.float32)
    wpad[0, :10] = np.asarray(w, dtype=np.float32).reshape(-1)
    X = np.asarray(X, dtype=np.float32)
    Xh = X.astype(np.float16)  # (a, b, c); cs innermost stays contiguous
    idx = np.arange(A)
    diag = X[idx, idx, :]  # (512, 1024) f32
    # td[a', ac, cs] with a = ac*128 + a'
    td_full = np.ascontiguousarray(diag.reshape(NAC, 128, C).transpose(1, 0, 2))
    in_maps = []
    for k in range(8):
        sl = slice(k * CS, (k + 1) * CS)
        in_maps.append({
            "x2": np.ascontiguousarray(Xh[:, :, sl]),
            "td": np.ascontiguousarray(td_full[:, :, sl]),
            "w": wpad,
        })
    res = run_bass_kernel_spmd(nc, in_maps, core_ids=list(range(8)), **kwargs)
    Y = np.concatenate([r["y"] for r in res.results], axis=0)
    return Y, res


def kernel(X: np.ndarray, weights: np.ndarray) -> np.ndarray:
    X = np.asarray(X, dtype=np.float32)
    Y, _ = _run(X, weights)
    return Y
